# revision 49
# baseline (speedup 1.0000x reference)
"""Trainium2 Bass kernel for BroadcastingSelfAttention.

Reference computation (see problem):
    score(s,b,t) = softplus(sum_f X[s,b,f,t] * W[s,f] + bias[s])
    w(d,s,b,t)   = softmax_s(-score(s,b,t) * dist(d,s))
    out(d,b,f,t) = sum_s w(d,s,b,t) * X[s,b,f,t]

Shapes: S=64, B=16, F=64, T=96, D=1024 (= 32*32 target grid).

Sharding: B=16 split across 8 cores (2 batches per core). Each core reads its
X slice + replicated dist/score, writes its out slice (25 MB bf16).

score is computed on the host (0.1% of total FLOPs: S*B*T*F MACs) so the
device does only the three heavy streams, one per engine class:
  * ScalarE: e2[(th,s)=128p, d=1024] = exp(score(s, 2*tp+th) * (-dist(d,s)))
    per t-pair (t-parity packs two t's into the 128 partitions; dist is
    pre-negated so the ACT per-partition `scale` operand carries +score)
  * TensorE: per round, 16 numerator matmuls (stationary e2[s=64, dblk=128
    cols], moving X[s,f,t], N=64 -> pm[d=128, f=64], 8 outs per psum bank,
    2 banks per round) + 16 denominator matmuls (same stationary, ones
    column, N=2) into denq, a separate psum tile holding 16 rounds of
    denominator columns contiguously.
  * VectorE: ONE fast-approx reciprocal per round-PAIR (32 denominators from
    denq — batching psum-side per-op overhead) + one fused normalize-drain
    TT (psum * recip -> bf16 stage) per round; every BOUNCE_PERIODth round
    drains via ScalarE-copy + Pool-multiply instead to balance the two
    bottleneck engines (ACT ~94%, DVE ~90% busy).
  * DMA: per filled stage quarter, 1 MiB transfers with 1 KiB runs,
    alternating SP/Pool queues; final quarter as two parallel eighths.

Note: a 3-deep psum rotation for pm (whether via a bufs=3 pool or two
alternating pools) passes CoreSim but crashes on hardware — keep pm
double-buffered.
"""

import numpy as np
import ml_dtypes

import concourse.bass as bass
import concourse.tile as tile
from concourse import bacc, mybir
from concourse import bass_utils

F32 = mybir.dt.float32
BF16 = mybir.dt.bfloat16

# Problem shapes (hardcoded per contract)
S = 64          # sources
B = 16          # total batch
NCORES = 8
BL = B // NCORES  # batches per core = 2
F = 64          # features
T = 96          # time
D = 1024        # flattened target grid 32*32
DBLK = D // 128  # 8 d-blocks of 128
TP = T // 2     # 48 t-pairs
TCH = 32        # t-chunk (stage tile holds 32 t values = 16 pairs)
NCH = T // TCH  # 3 chunks
RPC = TCH // 2  # 16 rounds (t-pairs) per chunk
PPC = RPC // 2  # 8 round-pairs per chunk

FP = F + 1      # 65: X feature columns + fused ones column
EP = 16         # rounds per denominator-psum epoch (16 rounds x 32 cols)

# OUT_DT: dtype of the staged/DMA'd output (host upcasts to f32). bf16 halves
# the dominant output DMA traffic at ~0.4% relative error.
OUT_DT = BF16
# Every Nth round bounces psum through SBUF via ScalarE-copy + GpSimd-
# normalize instead of the VectorE drain (0 = never). Offloads the DVE
# bottleneck without extra psum pressure.
BOUNCE_PERIOD = 10


def build_kernel():
    nc = bacc.Bacc("TRN2", target_bir_lowering=False, debug=False,
                   num_devices=NCORES)

    # xb[th, s, b, f', tp] = X[s, b, f, 2*tp+th] for f'<64, 1.0 at f'=64
    # (host pre-shuffled t-parity, pre-cast bf16, ones column fused)
    x_t = nc.dram_tensor("xb", (2, S, BL, FP, TP), BF16, kind="ExternalInput")
    # nds[(th,s), 0:D] = -dist[d, s] (host pre-transposed/negated, replicated
    # over t-parity); nds[(th,s), D + b*TP + tp] = host softplus score — one
    # combined tensor so startup pays a single DMA fixed cost
    nds_t = nc.dram_tensor("nds", (128, D + BL * TP), F32, kind="ExternalInput")
    # Output in hardware-native layout (host un-permutes). Ordered so one
    # DMA per (b, chunk) has 4 KiB-contiguous DRAM runs:
    # [b, dblk, p, ch, tlh, f, tlo] -> out[dblk*128+p, b, f, ch*TCH + tlh*8 + tlo]
    out_t = nc.dram_tensor("out_hw", (BL, DBLK, 128, NCH, 4, F, TCH // 4),
                           OUT_DT, kind="ExternalOutput")

    def dram_ap(t, offset, ap):
        base = t.ap()
        return bass.AP(tensor=base.tensor, offset=offset, ap=ap)

    with tile.TileContext(nc) as tc:
        with (
            tc.tile_pool(name="statics", bufs=1) as statics,
            tc.tile_pool(name="xin", bufs=2) as xin,
            tc.tile_pool(name="e2p", bufs=4) as e2p,
            tc.tile_pool(name="stage", bufs=3) as stagep,
            tc.tile_pool(name="small", bufs=4) as small,
            tc.tile_pool(name="bnc", bufs=2) as bncp,
            tc.tile_pool(name="psum", bufs=2, space="PSUM") as psump,
            tc.tile_pool(name="denp", bufs=2, space="PSUM") as denp,
        ):
            # ---- static tiles -------------------------------------------------
            # nds[(th,s), :]: cols 0:D = -dist, cols D: = score per (b, tp).
            # The sim charges a DMA's full duration to its issuing engine
            # queue, so startup-critical loads are spread across idle queues
            # (SP + DVE halves) and ACT stays clear for the exp stream.
            NSC = D + BL * TP
            nds = statics.tile([128, NSC], F32)
            for half, eng in ((0, nc.sync), (1, nc.scalar)):
                eng.dma_start(
                    out=nds[half * S : (half + 1) * S, :],
                    in_=dram_ap(nds_t, half * S * NSC, [[NSC, S], [1, NSC]]),
                )
            ndist2 = nds[:, 0:D]

            x2m_tiles = []
            for b in range(BL):
                # ---- x2m[(th,s)=128p, f'=65, tp=48] bf16 (ones fused) ----
                # (f', tp) is one contiguous 6240 B run per source row.
                # b0-th0 rides SP behind the nds half; the rest ride Pool
                # (SWDGE) so neither ACT nor the output chain is blocked.
                x2m = xin.tile([128, FP, TP], BF16)
                x2m_tiles.append(x2m)
                for th in range(2):
                    eng = nc.sync if (b == 0 and th == 0) else nc.gpsimd
                    eng.dma_start(
                        out=x2m[th * S : (th + 1) * S, :, :],
                        in_=dram_ap(
                            x_t, th * (S * BL * FP * TP) + b * (FP * TP),
                            [[BL * FP * TP, S], [1, FP * TP]],
                        ),
                    )

            denq = None
            for b in range(BL):
                x2m = x2m_tiles[b]

                for ch in range(NCH):
                    # stage[(d%128)=128p, dh=2, dl=4, tlh=4, f=64, tlo=8]
                    stage = stagep.tile([128, 2, 4, 4, F, TCH // 4], OUT_DT)
                    for pr in range(RPC // 2):
                        # ---- round pair: exps + matmuls ------------------
                        pms = []
                        for rr in range(2):
                            r = pr * 2 + rr
                            tp = ch * RPC + r
                            rg = (b * NCH + ch) * RPC + r
                            if rg % EP == 0:
                                # denq[(d%128), (rg%EP)*32 + (par*8+dblk)*2
                                # + 1]: denominator columns for a 16-round
                                # epoch, contiguous so reciprocals batch
                                denq = denp.tile([128, 512], F32, tag="denq")
                            # e2[(th,s), d] = exp(score * -dist)
                            e2 = e2p.tile([128, D], BF16)
                            nc.scalar.activation(
                                out=e2[:], in_=ndist2,
                                func=mybir.ActivationFunctionType.Exp,
                                scale=nds[:, D + b * TP + tp
                                          : D + b * TP + tp + 1],
                            )

                            # pm[128p=d, par=2, dh=2, dl=4, 64]: 2 banks; the
                            # denominator matmuls (ones column, N=2) go to
                            # denq with the same stationary
                            pm = psump.tile([128, 2, 2, 4, F], F32, tag="pm")
                            pms.append(pm)
                            for par in range(2):
                                p0 = par * S
                                for dh in range(2):
                                    for dl in range(4):
                                        dblk = dh * 4 + dl
                                        lhsT = e2[p0 : p0 + S,
                                                  dblk * 128 : (dblk + 1) * 128]
                                        nc.tensor.matmul(
                                            out=pm[:, par, dh, dl, :],
                                            lhsT=lhsT,
                                            rhs=x2m[p0 : p0 + S, 0:F, tp],
                                            start=True, stop=True,
                                        )
                                        # N=2 (cols F-1, F): col 0 is unused
                                        # filler, col 1 is the denominator
                                        dc = (rg % EP) * 32 \
                                            + (par * 8 + dblk) * 2
                                        nc.tensor.matmul(
                                            out=denq[:, dc : dc + 2],
                                            lhsT=lhsT,
                                            rhs=x2m[p0 : p0 + S, F - 1 : FP, tp],
                                            start=True, stop=True,
                                        )

                        # ---- one reciprocal per pair (32 denominators,
                        # ~51 ULP approx — den in [3e-3, 64]) --------------
                        rgl = (b * NCH + ch) * RPC + pr * 2
                        dbase = (rgl % EP) * 32 + 1
                        rc = small.tile([128, 2, 2, 2, 4], F32, tag="rc")
                        nc.vector.reciprocal_approx_fast(
                            out=rc[:].rearrange("p r a c e -> p (r a c) e"),
                            in_=denq[:, dbase : dbase + 63 : 2]
                                .rearrange("p (a e) -> p a e", a=8),
                        )

                        # ---- drain + normalize per round -----------------
                        for rr in range(2):
                            r = pr * 2 + rr
                            pm = pms[rr]
                            rg = rgl + rr
                            tlh, tlo = (2 * r) // 8, (2 * r) % 8
                            out_ap = stage[:, :, :, tlh, :,
                                           tlo : tlo + 2].rearrange(
                                "p a c e t -> p t a c e")
                            rc_b = rc[:, rr].unsqueeze(4).broadcast_to(
                                [128, 2, 2, 4, F])
                            # phase 3 keeps bounces away from the final
                            # rounds; the very last round bounces so its
                            # normalize runs parallel to round 94's drain
                            if BOUNCE_PERIOD and (
                                    rg % BOUNCE_PERIOD == 3
                                    or rg == BL * NCH * RPC - 1):
                                # psum -> SBUF on ScalarE, normalize on Pool
                                tmp = bncp.tile([128, 2, 2, 4, F], F32,
                                                tag="bnc")
                                nc.scalar.activation(
                                    out=tmp[:], in_=pm[:],
                                    func=mybir.ActivationFunctionType.Copy,
                                )
                                nc.gpsimd.tensor_tensor(
                                    out=out_ap,
                                    in0=tmp[:],
                                    in1=rc_b,
                                    op=mybir.AluOpType.mult,
                                )
                            else:
                                nc.vector.tensor_tensor(
                                    out=out_ap,
                                    in0=pm[:],
                                    in1=rc_b,
                                    op=mybir.AluOpType.mult,
                                )

                        # ---- DMA out each filled tlh quarter (1 MiB, 1 KiB
                        # runs): early starts keep the DMA chains smooth and
                        # the final transfer short. Quarters alternate between
                        # the SP and Pool queues (per-queue serialization).
                        # out_hw[b, dblk, p, ch, tlh, f, tlo] elem strides:
                        # tlo 1, f 8, tlh 512, ch 2048, p 6144, dblk 786432
                        r = pr * 2 + 1
                        tlh = (2 * r) // 8
                        if r % 4 == 3:
                            qi = (b * NCH + ch) * 4 + tlh
                            qofs = b * (DBLK * 128 * NCH * 2048) \
                                + ch * 2048 + tlh * 512
                            if qi == BL * NCH * 4 - 1:
                                # final quarter: two parallel eighths (SP +
                                # Pool) to halve the unoverlapped tail
                                for dh, out_eng in ((0, nc.sync),
                                                    (1, nc.gpsimd)):
                                    out_eng.dma_start(
                                        out=dram_ap(
                                            out_t,
                                            qofs + dh * (4 * 128 * NCH * 2048),
                                            [[NCH * 2048, 128],          # p
                                             [128 * NCH * 2048, 4],      # dl
                                             [1, 512]],                  # f,tlo
                                        ),
                                        in_=stage[:, dh, :, tlh, :, :],
                                    )
                            else:
                                out_eng = nc.sync if qi % 2 == 0 else nc.gpsimd
                                out_eng.dma_start(
                                    out=dram_ap(
                                        out_t, qofs,
                                        [[NCH * 2048, 128],           # p
                                         [4 * 128 * NCH * 2048, 2],   # dh
                                         [128 * NCH * 2048, 4],       # dl
                                         [1, 512]],                   # (f, tlo)
                                    ),
                                    in_=stage[:, :, :, tlh, :, :],
                                )

    nc.compile()
    return nc


_NC_CACHE = None


def _get_nc():
    global _NC_CACHE
    if _NC_CACHE is None:
        _NC_CACHE = build_kernel()
    return _NC_CACHE


def _softplus(z):
    # numerically stable ln(1+e^z)
    return np.logaddexp(0.0, z)


def _host_prep(X, dist, attention_weight, attention_bias):
    """Shared host-side preprocessing -> (xb_full, ndist2, score_h_full)."""
    X = np.asarray(X, dtype=np.float32)                                # (S,B,F,T)
    dist_np = np.asarray(dist, dtype=np.float32).reshape(-1, S)        # (D,S)
    # ndist2[(th,s), d] = -dist[d, s], replicated over parity
    ndist2 = np.tile(-dist_np.T, (2, 1))                               # (128,D)
    w_np = np.asarray(attention_weight, np.float32)
    bias_np = np.asarray(attention_bias, np.float32)

    # score on host: S*B*T*F MACs (~6M) — 0.1% of the kernel's FLOPs
    z = np.einsum("sbft,sf->sbt", X, w_np, optimize=True) + bias_np[:, None, None]
    score = _softplus(z)                                               # (S,B,T)
    # -> [(th,s), B, TP]
    score_h = np.ascontiguousarray(
        score.reshape(S, B, TP, 2).transpose(3, 0, 1, 2).reshape(128, B, TP))

    # xb[th, s, b, f', tp]: X with ones column fused, bf16
    xp = X.reshape(S, B, F, TP, 2).transpose(4, 0, 1, 2, 3)            # (2,S,B,F,TP)
    xb = np.empty((2, S, B, FP, TP), dtype=ml_dtypes.bfloat16)
    xb[:, :, :, 0:F, :] = xp.astype(ml_dtypes.bfloat16)
    xb[:, :, :, F, :] = np.float32(1.0)
    return xb, ndist2, score_h


def _nds_feed(ndist2, score_h, b0):
    """Combined [128, D + BL*TP] static input for one core's batch slice."""
    nds = np.empty((128, D + BL * TP), np.float32)
    nds[:, 0:D] = ndist2
    nds[:, D:] = score_h[:, b0 : b0 + BL].reshape(128, BL * TP)
    return nds


def core0_feeds(inputs):
    """(nc, feed-dict for core 0) — used by profile_sim.py / test.py sim."""
    xb, ndist2, score_h = _host_prep(
        inputs["X"], inputs["dist"],
        inputs["attention_weight"], inputs["attention_bias"])
    return _get_nc(), {
        "xb": np.ascontiguousarray(xb[:, :, 0:BL]),
        "nds": _nds_feed(ndist2, score_h, 0),
    }


def unpermute(hw, out_slice):
    """out_hw[b, dblk, p, ch, tlh, f, tlo] -> out[dblk*128+p, b, f, t]."""
    out_slice[:] = (
        np.asarray(hw).astype(np.float32)
        .transpose(1, 2, 0, 5, 3, 4, 6)
        .reshape(D, hw.shape[0], F, T)
    )


def kernel(X, dist, attention_weight, attention_bias):
    xb, ndist2, score_h = _host_prep(X, dist, attention_weight, attention_bias)

    nc = _get_nc()
    in_maps = []
    for c in range(NCORES):
        in_maps.append({
            "xb": np.ascontiguousarray(xb[:, :, c * BL : (c + 1) * BL]),
            "nds": _nds_feed(ndist2, score_h, c * BL),
        })
    res = bass_utils.run_bass_kernel_spmd(nc, in_maps, core_ids=list(range(NCORES)))
    out = np.empty((D, B, F, T), dtype=np.float32)
    for c in range(NCORES):
        unpermute(res.results[c]["out_hw"], out[:, c * BL : (c + 1) * BL])
    return out.reshape(32, 32, B, F, T)


# revision 62
# speedup vs baseline: 1.0062x; 1.0062x over previous
"""Trainium2 Bass kernel for BroadcastingSelfAttention.

Reference computation (see problem):
    score(s,b,t) = softplus(sum_f X[s,b,f,t] * W[s,f] + bias[s])
    w(d,s,b,t)   = softmax_s(-score(s,b,t) * dist(d,s))
    out(d,b,f,t) = sum_s w(d,s,b,t) * X[s,b,f,t]

Shapes: S=64, B=16, F=64, T=96, D=1024 (= 32*32 target grid).

Sharding: B=16 split across 8 cores (2 batches per core). Each core reads its
X slice + replicated dist/score, writes its out slice (25 MB bf16).

score is computed on the host (0.1% of total FLOPs: S*B*T*F MACs) so the
device does only the three heavy streams, one per engine class:
  * ScalarE: e2[(th,s)=128p, d=1024] = exp(score(s, 2*tp+th) * (-dist(d,s)))
    per t-pair (t-parity packs two t's into the 128 partitions; dist is
    pre-negated so the ACT per-partition `scale` operand carries +score)
  * TensorE: per round, 16 numerator matmuls (stationary e2[s=64, dblk=128
    cols], moving X[s,f,t], N=64 -> pm[d=128, f=64], 8 outs per psum bank,
    2 banks per round) + 16 denominator matmuls (same stationary, ones
    column, N=2) into denq, a separate psum tile holding 16 rounds of
    denominator columns contiguously.
  * VectorE: ONE fast-approx reciprocal per round-PAIR (32 denominators from
    denq — batching psum-side per-op overhead) + one fused normalize-drain
    TT (psum * recip -> bf16 stage) per round; every BOUNCE_PERIODth round
    drains via ScalarE-copy + Pool-multiply instead to balance the two
    bottleneck engines (ACT ~94%, DVE ~90% busy).
  * DMA: per filled stage quarter, 1 MiB transfers with 1 KiB runs,
    alternating SP/Pool queues; final quarter as two parallel eighths.

Note: a 3-deep psum rotation for pm (whether via a bufs=3 pool or two
alternating pools) passes CoreSim but crashes on hardware — keep pm
double-buffered.
"""

import numpy as np
import ml_dtypes

import concourse.bass as bass
import concourse.tile as tile
from concourse import bacc, mybir
from concourse import bass_utils

F32 = mybir.dt.float32
BF16 = mybir.dt.bfloat16

# Problem shapes (hardcoded per contract)
S = 64          # sources
B = 16          # total batch
NCORES = 8
BL = B // NCORES  # batches per core = 2
F = 64          # features
T = 96          # time
D = 1024        # flattened target grid 32*32
DBLK = D // 128  # 8 d-blocks of 128
TP = T // 2     # 48 t-pairs
TCH = 32        # t-chunk (stage tile holds 32 t values = 16 pairs)
NCH = T // TCH  # 3 chunks
RPC = TCH // 2  # 16 rounds (t-pairs) per chunk
PPC = RPC // 2  # 8 round-pairs per chunk

FP = F + 1      # 65: X feature columns + fused ones column
EP = 16         # rounds per denominator-psum epoch (16 rounds x 32 cols)

# OUT_DT: dtype of the staged/DMA'd output (host upcasts to f32). bf16 halves
# the dominant output DMA traffic at ~0.4% relative error.
OUT_DT = BF16
# Every Nth round bounces psum through SBUF via ScalarE-copy + GpSimd-
# normalize instead of the VectorE drain (0 = never). Offloads the DVE
# bottleneck without extra psum pressure.
BOUNCE_PERIOD = 10


def build_kernel():
    nc = bacc.Bacc("TRN2", target_bir_lowering=False, debug=False,
                   num_devices=NCORES)

    # xb[th, s, b, f', tp] = X[s, b, f, 2*tp+th] for f'<64, 1.0 at f'=64
    # (host pre-shuffled t-parity, pre-cast bf16, ones column fused)
    x_t = nc.dram_tensor("xb", (2, S, BL, FP, TP), BF16, kind="ExternalInput")
    # nds[(th,s), 0:D] = -dist[d, s] (host pre-transposed/negated, replicated
    # over t-parity); nds[(th,s), D + b*TP + tp] = host softplus score — one
    # combined tensor so startup pays a single DMA fixed cost
    nds_t = nc.dram_tensor("nds", (128, D + BL * TP), F32, kind="ExternalInput")
    # Output in hardware-native layout (host un-permutes). Ordered so one
    # DMA per (b, chunk) has 4 KiB-contiguous DRAM runs:
    # [b, dblk, p, ch, tlh, f, tlo] -> out[dblk*128+p, b, f, ch*TCH + tlh*8 + tlo]
    out_t = nc.dram_tensor("out_hw", (BL, DBLK, 128, NCH, 4, F, TCH // 4),
                           OUT_DT, kind="ExternalOutput")

    def dram_ap(t, offset, ap):
        base = t.ap()
        return bass.AP(tensor=base.tensor, offset=offset, ap=ap)

    with tile.TileContext(nc) as tc:
        with (
            tc.tile_pool(name="statics", bufs=1) as statics,
            tc.tile_pool(name="xin", bufs=2) as xin,
            tc.tile_pool(name="e2p", bufs=4) as e2p,
            tc.tile_pool(name="stage", bufs=3) as stagep,
            tc.tile_pool(name="small", bufs=4) as small,
            tc.tile_pool(name="bnc", bufs=2) as bncp,
            tc.tile_pool(name="psum", bufs=2, space="PSUM") as psump,
            tc.tile_pool(name="denp", bufs=2, space="PSUM") as denp,
        ):
            # ---- static tiles -------------------------------------------------
            # nds[(th,s), :]: cols 0:D = -dist, cols D: = score per (b, tp).
            # The sim charges a DMA's full duration to its issuing engine
            # queue, so startup-critical loads are spread across idle queues
            # (SP + DVE halves) and ACT stays clear for the exp stream.
            NSC = D + BL * TP
            nds = statics.tile([128, NSC], F32)
            for half, eng in ((0, nc.sync), (1, nc.scalar)):
                eng.dma_start(
                    out=nds[half * S : (half + 1) * S, :],
                    in_=dram_ap(nds_t, half * S * NSC, [[NSC, S], [1, NSC]]),
                )
            ndist2 = nds[:, 0:D]

            x2m_tiles = []
            for b in range(BL):
                # ---- x2m[(th,s)=128p, f'=65, tp=48] bf16 (ones fused) ----
                # (f', tp) is one contiguous 6240 B run per source row.
                # b0-th0 rides SP behind the nds half; the rest ride Pool
                # (SWDGE) so neither ACT nor the output chain is blocked.
                x2m = xin.tile([128, FP, TP], BF16)
                x2m_tiles.append(x2m)
                for th in range(2):
                    eng = nc.sync if (b == 0 and th == 0) else nc.gpsimd
                    eng.dma_start(
                        out=x2m[th * S : (th + 1) * S, :, :],
                        in_=dram_ap(
                            x_t, th * (S * BL * FP * TP) + b * (FP * TP),
                            [[BL * FP * TP, S], [1, FP * TP]],
                        ),
                    )

            denq = None
            for b in range(BL):
                x2m = x2m_tiles[b]

                for ch in range(NCH):
                    # stage[(d%128)=128p, dh=2, dl=4, tlh=4, f=64, tlo=8]
                    stage = stagep.tile([128, 2, 4, 4, F, TCH // 4], OUT_DT)
                    for pr in range(RPC // 2):
                        # ---- round pair: exps + matmuls ------------------
                        pms = []
                        for rr in range(2):
                            r = pr * 2 + rr
                            tp = ch * RPC + r
                            rg = (b * NCH + ch) * RPC + r
                            if rg % EP == 0:
                                # denq[(d%128), (rg%EP)*32 + (par*8+dblk)*2
                                # + 1]: denominator columns for a 16-round
                                # epoch, contiguous so reciprocals batch
                                denq = denp.tile([128, 512], F32, tag="denq")
                            # e2[(th,s), d] = exp(score * -dist)
                            e2 = e2p.tile([128, D], BF16)
                            nc.scalar.activation(
                                out=e2[:], in_=ndist2,
                                func=mybir.ActivationFunctionType.Exp,
                                scale=nds[:, D + b * TP + tp
                                          : D + b * TP + tp + 1],
                            )

                            # pm[128p=d, par=2, dh=2, dl=4, 64]: 2 banks; the
                            # denominator matmuls (ones column, N=2) go to
                            # denq with the same stationary
                            pm = psump.tile([128, 2, 2, 4, F], F32, tag="pm")
                            pms.append(pm)
                            for par in range(2):
                                p0 = par * S
                                for dh in range(2):
                                    for dl in range(4):
                                        dblk = dh * 4 + dl
                                        lhsT = e2[p0 : p0 + S,
                                                  dblk * 128 : (dblk + 1) * 128]
                                        nc.tensor.matmul(
                                            out=pm[:, par, dh, dl, :],
                                            lhsT=lhsT,
                                            rhs=x2m[p0 : p0 + S, 0:F, tp],
                                            start=True, stop=True,
                                        )
                                        # N=2 (cols F-1, F): col 0 is unused
                                        # filler, col 1 is the denominator
                                        dc = (rg % EP) * 32 \
                                            + (par * 8 + dblk) * 2
                                        nc.tensor.matmul(
                                            out=denq[:, dc : dc + 2],
                                            lhsT=lhsT,
                                            rhs=x2m[p0 : p0 + S, F - 1 : FP, tp],
                                            start=True, stop=True,
                                        )

                        # ---- one reciprocal per pair (32 denominators,
                        # ~51 ULP approx — den in [3e-3, 64]) --------------
                        rgl = (b * NCH + ch) * RPC + pr * 2
                        dbase = (rgl % EP) * 32 + 1
                        rc = small.tile([128, 2, 2, 2, 4], F32, tag="rc")
                        nc.vector.reciprocal_approx_fast(
                            out=rc[:].rearrange("p r a c e -> p (r a c) e"),
                            in_=denq[:, dbase : dbase + 63 : 2]
                                .rearrange("p (a e) -> p a e", a=8),
                        )

                        # ---- drain + normalize per round -----------------
                        for rr in range(2):
                            r = pr * 2 + rr
                            pm = pms[rr]
                            rg = rgl + rr
                            tlh, tlo = (2 * r) // 8, (2 * r) % 8
                            out_ap = stage[:, :, :, tlh, :,
                                           tlo : tlo + 2].rearrange(
                                "p a c e t -> p t a c e")
                            rc_b = rc[:, rr].unsqueeze(4).broadcast_to(
                                [128, 2, 2, 4, F])
                            # phase 3 keeps bounces away from the final
                            # rounds; the very last round bounces so its
                            # normalize runs parallel to round 94's drain
                            if BOUNCE_PERIOD and (
                                    rg % BOUNCE_PERIOD == 3
                                    or rg == BL * NCH * RPC - 1):
                                # psum -> SBUF on ScalarE, normalize on Pool
                                tmp = bncp.tile([128, 2, 2, 4, F], F32,
                                                tag="bnc")
                                nc.scalar.activation(
                                    out=tmp[:], in_=pm[:],
                                    func=mybir.ActivationFunctionType.Copy,
                                )
                                nc.gpsimd.tensor_tensor(
                                    out=out_ap,
                                    in0=tmp[:],
                                    in1=rc_b,
                                    op=mybir.AluOpType.mult,
                                )
                            else:
                                nc.vector.tensor_tensor(
                                    out=out_ap,
                                    in0=pm[:],
                                    in1=rc_b,
                                    op=mybir.AluOpType.mult,
                                )

                        # ---- DMA out each filled tlh quarter (1 MiB, 1 KiB
                        # runs): early starts keep the DMA chains smooth and
                        # the final transfer short. Quarters alternate between
                        # the SP and Pool queues (per-queue serialization).
                        # out_hw[b, dblk, p, ch, tlh, f, tlo] elem strides:
                        # tlo 1, f 8, tlh 512, ch 2048, p 6144, dblk 786432
                        r = pr * 2 + 1
                        tlh = (2 * r) // 8
                        if r % 4 == 3:
                            qi = (b * NCH + ch) * 4 + tlh
                            qofs = b * (DBLK * 128 * NCH * 2048) \
                                + ch * 2048 + tlh * 512
                            if qi == BL * NCH * 4 - 1:
                                # final quarter: two parallel eighths (SP +
                                # Pool) to halve the unoverlapped tail
                                for dh, out_eng in ((0, nc.sync),
                                                    (1, nc.gpsimd)):
                                    out_eng.dma_start(
                                        out=dram_ap(
                                            out_t,
                                            qofs + dh * (4 * 128 * NCH * 2048),
                                            [[NCH * 2048, 128],          # p
                                             [128 * NCH * 2048, 4],      # dl
                                             [1, 512]],                  # f,tlo
                                        ),
                                        in_=stage[:, dh, :, tlh, :, :],
                                    )
                            else:
                                out_eng = nc.sync if qi % 2 == 0 else nc.gpsimd
                                out_eng.dma_start(
                                    out=dram_ap(
                                        out_t, qofs,
                                        [[NCH * 2048, 128],           # p
                                         [4 * 128 * NCH * 2048, 2],   # dh
                                         [128 * NCH * 2048, 4],       # dl
                                         [1, 512]],                   # (f, tlo)
                                    ),
                                    in_=stage[:, :, :, tlh, :, :],
                                )

    nc.compile()
    return nc


_NC_CACHE = None


def _get_nc():
    global _NC_CACHE
    if _NC_CACHE is None:
        _NC_CACHE = build_kernel()
    return _NC_CACHE


def _softplus(z):
    # numerically stable ln(1+e^z)
    return np.logaddexp(0.0, z)


def _host_prep(X, dist, attention_weight, attention_bias):
    """Shared host-side preprocessing -> (xb_full, ndist2, score_h_full)."""
    X = np.asarray(X, dtype=np.float32)                                # (S,B,F,T)
    dist_np = np.asarray(dist, dtype=np.float32).reshape(-1, S)        # (D,S)
    # ndist2[(th,s), d] = -dist[d, s], replicated over parity
    ndist2 = np.tile(-dist_np.T, (2, 1))                               # (128,D)
    w_np = np.asarray(attention_weight, np.float32)
    bias_np = np.asarray(attention_bias, np.float32)

    # score on host: S*B*T*F MACs (~6M) — 0.1% of the kernel's FLOPs
    z = np.einsum("sbft,sf->sbt", X, w_np, optimize=True) + bias_np[:, None, None]
    score = _softplus(z)                                               # (S,B,T)
    # -> [(th,s), B, TP]
    score_h = np.ascontiguousarray(
        score.reshape(S, B, TP, 2).transpose(3, 0, 1, 2).reshape(128, B, TP))

    # xb[th, s, b, f', tp]: X with ones column fused, bf16
    xp = X.reshape(S, B, F, TP, 2).transpose(4, 0, 1, 2, 3)            # (2,S,B,F,TP)
    xb = np.empty((2, S, B, FP, TP), dtype=ml_dtypes.bfloat16)
    xb[:, :, :, 0:F, :] = xp.astype(ml_dtypes.bfloat16)
    xb[:, :, :, F, :] = np.float32(1.0)
    return xb, ndist2, score_h


def _nds_feed(ndist2, score_h, b0):
    """Combined [128, D + BL*TP] static input for one core's batch slice."""
    nds = np.empty((128, D + BL * TP), np.float32)
    nds[:, 0:D] = ndist2
    nds[:, D:] = score_h[:, b0 : b0 + BL].reshape(128, BL * TP)
    return nds


def core0_feeds(inputs):
    """(nc, feed-dict for core 0) — used by profile_sim.py / test.py sim."""
    xb, ndist2, score_h = _host_prep(
        inputs["X"], inputs["dist"],
        inputs["attention_weight"], inputs["attention_bias"])
    return _get_nc(), {
        "xb": np.ascontiguousarray(xb[:, :, 0:BL]),
        "nds": _nds_feed(ndist2, score_h, 0),
    }


def unpermute(hw, out_slice):
    """out_hw[b, dblk, p, ch, tlh, f, tlo] -> out[dblk*128+p, b, f, t]."""
    out_slice[:] = (
        np.asarray(hw).astype(np.float32)
        .transpose(1, 2, 0, 5, 3, 4, 6)
        .reshape(D, hw.shape[0], F, T)
    )


def kernel(X, dist, attention_weight, attention_bias):
    xb, ndist2, score_h = _host_prep(X, dist, attention_weight, attention_bias)

    nc = _get_nc()
    in_maps = []
    for c in range(NCORES):
        in_maps.append({
            "xb": np.ascontiguousarray(xb[:, :, c * BL : (c + 1) * BL]),
            "nds": _nds_feed(ndist2, score_h, c * BL),
        })
    res = bass_utils.run_bass_kernel_spmd(nc, in_maps, core_ids=list(range(NCORES)))
    out = np.empty((D, B, F, T), dtype=np.float32)
    for c in range(NCORES):
        unpermute(res.results[c]["out_hw"], out[:, c * BL : (c + 1) * BL])
    return out.reshape(32, 32, B, F, T)
